# revision 20
# baseline (speedup 1.0000x reference)
"""CoPE multi-head attention Trainium2 kernel (v2).

Sharding: 16 heads / 8 cores = 2 heads per core (head/tensor parallel).
Each core gets full q,k,v (host-transposed, fp16) + its head-slice of the
projection weights, computes both heads' attention and the summed output
projection partial; host sums the 8 fp16 partials and adds the output bias.

CoPE structure exploited: pos = reverse-cumsum of sigmoid gates clips at
npos-1=63.  For keys before a 256-wide tail suffix, pos >= 63 (verified at
runtime via a flag output), so cope == T[q,63], a per-row constant that
cancels in softmax.  Only the tail needs the real interpolated gather,
done via GPSIMD local_scatter (run-start positions -> table values) + a
sample-and-hold tensor_tensor_scan.

v2 vs v1: fp16 data everywhere (half DMA, FWL weight loads, DVE 2x),
row-tile-packed 64-contract matmuls (both heads concurrent in the PE
array), merged two-head tail chains (half the DVE op count), q-halved
k-loop fitting PSUM, on-device head-sum + fp16 output (1/4 the output
DMA), engine rebalancing (exp stream owns ACT, chains own DVE/GPSIMD).
"""

import numpy as np

B, S, ND, NH, DH, NPOS = 1, 2048, 1024, 16, 64, 64
NCORES = 8
HPC = NH // NCORES          # heads per core = 2
DHC = HPC * DH              # head dims per core = 128
TAIL = 256                  # tail width (suffix of key axis)
KHEAD = S - TAIL            # 1792
NQB = S // 128              # 16 q blocks
NKB = S // 128              # 16 k blocks
NKB_HEAD = KHEAD // 128     # 14
GRP = 4                     # q-blocks per merged scatter group
NG = NQB // GRP             # 4 groups
TAILC = 160                 # columns of the tail that get the full CoPE chain
TAIL0 = TAIL - TAILC        # leading tail columns treated as clipped
SQH = S // 2                # q-half width = 1024

_prog = None


def _build_program():
    import concourse.bacc as bacc
    import concourse.tile as tile
    from concourse import mybir

    dt = mybir.dt
    AF = mybir.ActivationFunctionType
    OP = mybir.AluOpType
    f16 = dt.float16
    f32 = dt.float32

    nc = bacc.Bacc("TRN2", target_bir_lowering=False, debug=False,
                   num_devices=NCORES)

    # ---- DRAM I/O ----
    qT_d = nc.dram_tensor("qT", [ND, S], f16, kind="ExternalInput").ap()
    kT_d = nc.dram_tensor("kT", [ND, S], f16, kind="ExternalInput").ap()
    vT_d = nc.dram_tensor("vT", [ND, S], f16, kind="ExternalInput").ap()
    # host pre-arranges projection weights as [128, 8, 128] (partition-major)
    # so the DMA is one contiguous 2KB line per partition
    wqT_d = nc.dram_tensor("wqT", [128, 8 * DHC], f16, kind="ExternalInput").ap()
    wkT_d = nc.dram_tensor("wkT", [128, 8 * DHC], f16, kind="ExternalInput").ap()
    wvT_d = nc.dram_tensor("wvT", [128, 8 * DHC], f16, kind="ExternalInput").ap()
    woT_d = nc.dram_tensor("woT", [DHC, ND], f16, kind="ExternalInput").ap()
    bq_d = nc.dram_tensor("bq", [DHC, 1], f32, kind="ExternalInput").ap()
    bk_d = nc.dram_tensor("bk", [DHC, 1], f32, kind="ExternalInput").ap()  # pre-scaled 1/8
    bv_d = nc.dram_tensor("bv", [DHC, 1], f32, kind="ExternalInput").ap()
    pe_d = nc.dram_tensor("pe", [2 * DH, NPOS], f16, kind="ExternalInput").ap()
    iota_d = nc.dram_tensor("iota", [128, HPC * GRP * TAILC], dt.int16,
                            kind="ExternalInput").ap()
    off_d = nc.dram_tensor("off", [128, HPC, GRP, TAILC], f16,
                           kind="ExternalInput").ap()
    rmask_d = nc.dram_tensor("rmask", [128, HPC, GRP, TAILC], f16,
                             kind="ExternalInput").ap()
    identh_d = nc.dram_tensor("identh", [128, 128], f16, kind="ExternalInput").ap()
    out_d = nc.dram_tensor("out", [S, ND], f16, kind="ExternalOutput").ap()
    flag_d = nc.dram_tensor("flag", [128, HPC * NQB], f32,
                            kind="ExternalOutput").ap()

    with tile.TileContext(nc) as tc:
        consts = tc.alloc_tile_pool(name="consts", bufs=1)
        big = tc.alloc_tile_pool(name="big", bufs=1)

        # ---- constants ----
        wqT = consts.tile([128, 8, DHC], f16, tag="wq")
        wkT = consts.tile([128, 8, DHC], f16, tag="wk")
        wvT = consts.tile([128, 8, DHC], f16, tag="wv")
        nc.sync.dma_start(out=wqT, in_=wqT_d.rearrange("p (b d) -> p b d", b=8))
        nc.sync.dma_start(out=wkT, in_=wkT_d.rearrange("p (b d) -> p b d", b=8))
        nc.sync.dma_start(out=wvT, in_=wvT_d.rearrange("p (b d) -> p b d", b=8))
        woT = [consts.tile([DH, ND], f16, tag=f"wo{h}", name=f"woT{h}")
               for h in range(HPC)]
        for h in range(HPC):
            nc.sync.dma_start(out=woT[h], in_=woT_d[h * DH:(h + 1) * DH, :])
        bq = consts.tile([DHC, 1], f32, tag="bq")
        bk = consts.tile([DHC, 1], f32, tag="bk")
        bv = consts.tile([DHC, 1], f32, tag="bv")
        nc.sync.dma_start(out=bq, in_=bq_d)
        nc.sync.dma_start(out=bk, in_=bk_d)
        nc.sync.dma_start(out=bv, in_=bv_d)
        pe = consts.tile([2 * DH, NPOS], f16, tag="pe")
        nc.sync.dma_start(out=pe, in_=pe_d)
        iota = consts.tile([128, HPC * GRP * TAILC], dt.int16, tag="iota")
        nc.sync.dma_start(out=iota, in_=iota_d)
        offt = consts.tile([128, HPC, GRP, TAILC], f16, tag="offt")
        nc.sync.dma_start(out=offt, in_=off_d)
        rmask = consts.tile([128, HPC, GRP, TAILC], f16, tag="rmask")
        nc.sync.dma_start(out=rmask, in_=rmask_d)
        identh = consts.tile([128, 128], f16, tag="identh")
        nc.sync.dma_start(out=identh, in_=identh_d)

        # ---- persistent activations ----
        QT = big.tile([DHC, S], f16, tag="QT")   # h0 rows 0:64, h1 rows 64:128
        KT = big.tile([DHC, S], f16, tag="KT")   # pre-scaled by 1/8
        # per kblk: h0 V[0:64], ones col 64, h1 V cols 66:130, ones col 130
        Vn = big.tile([128, NKB, 131], f16, tag="Vn")
        # cope tables, group-major so a merged group slice is contiguous
        Tp = big.tile([128, NG, HPC, GRP, NPOS], f16, tag="Tp")
        dTp = big.tile([128, NG, HPC, GRP, NPOS], f16, tag="dTp")
        # raw tail logits, then logits+cope (pre-exp)
        ttmp = big.tile([128, HPC, NQB, TAIL], f16, tag="ttmp")
        # transposed tail exp-scores per head: [t-block, qb, q]
        sct = [big.tile([128, 2, NQB, 128], f16, tag=f"sct{h}", name=f"sct{h}")
               for h in range(HPC)]
        hoT = [big.tile([65, S], f16, tag=f"hoT{h}", name=f"hoT{h}")
               for h in range(HPC)]
        rden = big.tile([128, HPC, NQB], f32, tag="rden")
        flag_sb = big.tile([128, HPC, NQB], f32, tag="flag")

        with tc.tile_pool(name="xstage", bufs=3) as xstage, \
             tc.tile_pool(name="proj_ps", bufs=1, space="PSUM") as proj_ps, \
             tc.tile_pool(name="tl1", bufs=1) as tl1, \
             tc.tile_pool(name="tl2", bufs=2) as tl2, \
             tc.tile_pool(name="tl3", bufs=3) as tl3, \
             tc.tile_pool(name="tls", bufs=4) as tls:

            # ---- phase 1: Q,K projections ----
            for name, x_d, wT, bias, scale in (
                ("q", qT_d, wqT, bq, 1.0),
                ("k", kT_d, wkT, bk, 0.125),
            ):
                dest = QT if name == "q" else KT
                ps_chunks = [proj_ps.tile([DHC, 512], f32, tag=f"pp{c}",
                                          name=f"pp_{name}_{c}") for c in range(4)]
                for db in range(8):
                    xt = xstage.tile([128, S], f16, tag="xt", name=f"xt_{name}_{db}")
                    nc.sync.dma_start(out=xt, in_=x_d[db * 128:(db + 1) * 128, :])
                    for c in range(4):
                        nc.tensor.matmul(out=ps_chunks[c], lhsT=wT[:, db, :],
                                         rhs=xt[:, c * 512:(c + 1) * 512],
                                         start=(db == 0), stop=(db == 7))
                for c in range(4):
                    nc.scalar.activation(out=dest[:, c * 512:(c + 1) * 512],
                                         in_=ps_chunks[c], func=AF.Identity,
                                         bias=bias, scale=scale)

            # ---- phase 2: cope tables T'[q,n], dT'[q,n] (fp16), packed heads ----
            with tc.tile_pool(name="tt_ps", bufs=2, space="PSUM") as tt_ps, \
                 tc.tile_pool(name="tsbp", bufs=2) as tsbp:
                for gg in range(2):
                    ps = tt_ps.tile([128, HPC, 8, NPOS], f32, tag="ttp")
                    for qq in range(8):
                        qb = gg * 8 + qq
                        for h in range(HPC):
                            nc.tensor.matmul(
                                out=ps[:, h, qq, :],
                                lhsT=QT[h * DH:(h + 1) * DH, qb * 128:(qb + 1) * 128],
                                rhs=pe[h * DH:(h + 1) * DH, :],
                                start=True, stop=True)
                    tsb = tsbp.tile([128, HPC, 8, NPOS], f32, tag="tsb")
                    nc.vector.tensor_copy(out=tsb, in_=ps)
                    for qq in range(8):
                        qb = gg * 8 + qq
                        g, j = qb // GRP, qb % GRP
                        for h in range(HPC):
                            nc.vector.tensor_scalar(
                                out=Tp[:, g, h, j, :], in0=tsb[:, h, qq, :],
                                scalar1=tsb[:, h, qq, 63:64], scalar2=None,
                                op0=OP.subtract)
                    for h in range(HPC):
                        nc.vector.tensor_tensor(
                            out=dTp[:, gg * 2:(gg + 1) * 2, h, :, 0:63],
                            in0=tsb[:, h, :, 1:64].rearrange(
                                "p (g j) n -> p g j n", g=2),
                            in1=tsb[:, h, :, 0:63].rearrange(
                                "p (g j) n -> p g j n", g=2),
                            op=OP.subtract)
                nc.vector.memset(dTp[:, :, :, :, 63:64], 0.0)

            # ---- phase 3: tail QK (packed heads) -> raw logits into ttmp ----
            with tc.tile_pool(name="st_ps", bufs=1, space="PSUM") as st_ps:
                for qb in range(NQB):
                    stp = st_ps.tile([128, HPC, 512], f32, tag="stp")
                    for h in range(HPC):
                        nc.tensor.matmul(
                            out=stp[:, h, 0:TAIL],
                            lhsT=QT[h * DH:(h + 1) * DH, qb * 128:(qb + 1) * 128],
                            rhs=KT[h * DH:(h + 1) * DH, KHEAD:S],
                            start=True, stop=True)
                    nc.scalar.activation(out=ttmp[:, :, qb, :],
                                         in_=stp[:, :, 0:TAIL], func=AF.Identity)

            # ---- phase 4: merged two-head tail cope chains ----
            def chain_pre(g):
                gts = tl2.tile([128, HPC, GRP, TAILC], f16, tag="gts",
                               name=f"gts{g}")
                posb = tl2.tile([128, HPC, GRP, TAILC], f32, tag="posb",
                                name=f"posb{g}")
                wb = tl3.tile([128, HPC, GRP, TAILC], f16, tag="wb",
                              name=f"wb{g}")
                c16 = tl2.tile([128, HPC, GRP, TAILC], dt.int16, tag="c16",
                               name=f"c16{g}")
                cfb = tl2.tile([128, HPC, GRP, TAILC], f16, tag="cfb",
                               name=f"cfb{g}")
                ieq = tl3.tile([128, HPC, GRP, TAILC], f16, tag="ieq",
                               name=f"ieq{g}")
                mi16 = tl3.tile([128, HPC, GRP, TAILC], dt.int16, tag="mi16",
                                name=f"mi16{g}")
                gsb = tls.tile([128, HPC, GRP], f32, tag="gsb", name=f"gsb{g}")
                spsc = ttmp[:, :, g * GRP:(g + 1) * GRP, TAIL0:TAIL]
                for h in range(HPC):
                    for j in range(GRP):
                        nc.scalar.activation(
                            out=gts[:, h, j, :],
                            in_=ttmp[:, h, g * GRP + j, TAIL0:TAIL],
                            func=AF.Sigmoid, accum_out=gsb[:, h, j:j + 1])
                nc.vector.tensor_copy(out=flag_sb[:, :, g * GRP:(g + 1) * GRP],
                                      in_=gsb)
                # col0 of each sub-block: g[0] - gsum (seeds the chained scan)
                nc.vector.tensor_tensor(
                    out=gts[:, :, :, 0:1], in0=gts[:, :, :, 0:1],
                    in1=gsb.rearrange("p h (g o) -> p h g o", o=1), op=OP.subtract)
                # chained cumsum with reset: state = rmask*state + g'
                nc.vector.tensor_tensor_scan(
                    out=posb.rearrange("p h g t -> p (h g t)"),
                    data0=rmask.rearrange("p h g t -> p (h g t)"),
                    data1=gts.rearrange("p h g t -> p (h g t)"), initial=0.0,
                    op0=OP.mult, op1=OP.add)
                # restore col0, then pos = g - (cs - gsum)
                nc.vector.tensor_tensor(
                    out=gts[:, :, :, 0:1], in0=gts[:, :, :, 0:1],
                    in1=gsb.rearrange("p h (g o) -> p h g o", o=1), op=OP.add)
                nc.vector.scalar_tensor_tensor(out=posb, in0=posb, scalar=-1.0,
                                               in1=gts, op0=OP.mult, op1=OP.add)
                nc.vector.tensor_scalar(out=c16, in0=posb, scalar1=0.5,
                                        scalar2=62.0, op0=OP.subtract, op1=OP.min)
                nc.vector.tensor_copy(out=cfb, in_=c16)
                nc.vector.scalar_tensor_tensor(out=wb, in0=posb, scalar=63.0,
                                               in1=cfb, op0=OP.min, op1=OP.subtract)
                # ieq = 1 where floor-index unchanged (keep-state mask for the
                # sample-and-hold scans); 0 at run starts.  col0 forced run-start.
                nc.vector.tensor_tensor(out=ieq[:, :, :, 1:], in0=cfb[:, :, :, 1:],
                                        in1=cfb[:, :, :, :-1], op=OP.is_equal)
                nc.vector.memset(ieq[:, :, :, 0:1], 0.0)
                # scatter index: cfb+off at run starts, pushed negative elsewhere
                nc.vector.tensor_add(out=cfb, in0=cfb, in1=offt)
                nc.vector.scalar_tensor_tensor(out=mi16, in0=ieq, scalar=-1024.0,
                                               in1=cfb, op0=OP.mult, op1=OP.add)
                return dict(gts=gts, posb=posb, wb=wb, c16=c16, cfb=cfb, ieq=ieq,
                            mi16=mi16, gsb=gsb, spsc=spsc)

            def chain_scatter1(g, st):
                ptab = tl3.tile([128, HPC * GRP * NPOS], dt.int16, tag="ptab",
                                name=f"ptab{g}")
                nc.gpsimd.local_scatter(out_ap=ptab[:], data_ap=iota[:],
                                        idxs_ap=st['mi16'][:], channels=128,
                                        num_elems=HPC * GRP * NPOS,
                                        num_idxs=HPC * GRP * TAILC)
                st['ptab'] = ptab

            def chain_scatter23(g, st):
                ptab = st['ptab']
                scA = tl3.tile([128, HPC, GRP, TAILC], f16, tag="scA",
                               name=f"scA{g}")
                scB = tl3.tile([128, HPC, GRP, TAILC], f16, tag="scB",
                               name=f"scB{g}")
                nc.vector.tensor_scalar(out=ptab, in0=ptab, scalar1=1,
                                        scalar2=None, op0=OP.subtract)
                nc.gpsimd.local_scatter(out_ap=scA[:], data_ap=Tp[:, g],
                                        idxs_ap=ptab[:], channels=128,
                                        num_elems=HPC * GRP * TAILC,
                                        num_idxs=HPC * GRP * NPOS)
                nc.gpsimd.local_scatter(out_ap=scB[:], data_ap=dTp[:, g],
                                        idxs_ap=ptab[:], channels=128,
                                        num_elems=HPC * GRP * TAILC,
                                        num_idxs=HPC * GRP * NPOS)
                st['scA'] = scA
                st['scB'] = scB

            def chain_post(g, st):
                Ab = tl1.tile([128, HPC, GRP, TAILC], f16, tag="Ab",
                              name=f"Ab{g}")
                Bb = tl1.tile([128, HPC, GRP, TAILC], f16, tag="Bb",
                              name=f"Bb{g}")
                # col0 of every sub-block is a forced run-start, so one chained
                # scan over the flattened group self-resets at block boundaries
                nc.vector.tensor_tensor_scan(
                    out=Ab.rearrange("p h g t -> p (h g t)"),
                    data0=st['ieq'].rearrange("p h g t -> p (h g t)"),
                    data1=st['scA'].rearrange("p h g t -> p (h g t)"), initial=0.0,
                    op0=OP.mult, op1=OP.add)
                nc.vector.tensor_tensor_scan(
                    out=Bb.rearrange("p h g t -> p (h g t)"),
                    data0=st['ieq'].rearrange("p h g t -> p (h g t)"),
                    data1=st['scB'].rearrange("p h g t -> p (h g t)"), initial=0.0,
                    op0=OP.mult, op1=OP.add)
                nc.gpsimd.tensor_tensor(out=Bb, in0=st['wb'], in1=Bb,
                                        op=OP.mult)
                nc.gpsimd.tensor_tensor(out=Ab, in0=Ab, in1=Bb, op=OP.add)
                nc.vector.tensor_add(out=st['spsc'], in0=st['spsc'], in1=Ab)

            # software-pipelined emission: the DVE queue is in-order, so
            # interleave the next groups' pre-work between the GPSIMD scatter
            # calls to keep both engines busy (a cross-engine wait parked at
            # the DVE queue head would otherwise idle it for the whole scatter)
            sts = [None] * NG
            sts[0] = chain_pre(0)
            chain_scatter1(0, sts[0])
            if NG > 1:
                sts[1] = chain_pre(1)
            for g in range(NG):
                chain_scatter23(g, sts[g])
                if g + 1 < NG:
                    chain_scatter1(g + 1, sts[g + 1])
                if g + 2 < NG:
                    sts[g + 2] = chain_pre(g + 2)
                chain_post(g, sts[g])

            # ---- phase 5: V projection + transpose-pack into Vn ----
            with tc.tile_pool(name="vt_ps", bufs=2, space="PSUM") as vt_ps, \
                 tc.tile_pool(name="vstage", bufs=1) as vstage:
                VT = vstage.tile([DHC, S], f16, tag="VT")
                ps_chunks = [proj_ps.tile([DHC, 512], f32, tag=f"pp{c}",
                                          name=f"pp_v_{c}") for c in range(4)]
                for db in range(8):
                    xt = xstage.tile([128, S], f16, tag="xt", name=f"xt_v_{db}")
                    nc.sync.dma_start(out=xt, in_=vT_d[db * 128:(db + 1) * 128, :])
                    for c in range(4):
                        nc.tensor.matmul(out=ps_chunks[c], lhsT=wvT[:, db, :],
                                         rhs=xt[:, c * 512:(c + 1) * 512],
                                         start=(db == 0), stop=(db == 7))
                for c in range(4):
                    nc.scalar.activation(out=VT[:, c * 512:(c + 1) * 512],
                                         in_=ps_chunks[c], func=AF.Identity,
                                         bias=bv)
                nc.vector.memset(Vn[:, :, 64:65], 1.0)
                nc.vector.memset(Vn[:, :, 130:131], 1.0)
                for kb in range(NKB):
                    pt = vt_ps.tile([128, 128], f16, tag="vt")
                    nc.tensor.transpose(pt, VT[:, kb * 128:(kb + 1) * 128], identh)
                    dst = bass_ap_2range(Vn, kb)
                    nc.scalar.activation(out=dst,
                                         in_=pt.rearrange("p (b d) -> p b d", b=2),
                                         func=AF.Identity)

        # ---- phases 6-9: k-loop (both heads packed, q-halved), tail exp +
        # transpose, denominators, output projection with on-device head-sum ----
        # One shared 2-slot PSUM pool (tag "sp", [128,2,512]f32-sized = 2 banks
        # per slot) serves the k-loop QK tiles, the tail transposes, the
        # denominator transposes and the Wo accumulators: pv(4) + shared(4) = 8.
        with tc.tile_pool(name="pv_ps", bufs=1, space="PSUM") as pv_ps, \
             tc.tile_pool(name="sh_ps", bufs=2, space="PSUM") as sh_ps, \
             tc.tile_pool(name="sc_pool", bufs=3) as sc_pool, \
             tc.tile_pool(name="et", bufs=2) as et, \
             tc.tile_pool(name="ob_pool", bufs=2) as ob_pool, \
             tc.tile_pool(name="dc_pool", bufs=2) as dc_pool:

            def vrange(kb, h):
                return Vn[:, kb, 0:65] if h == 0 else Vn[:, kb, 66:131]

            def pv_mm(pvs, kb, rhs_fn, start, stop):
                for h in range(HPC):
                    for c in range(2):
                        nc.tensor.matmul(out=pvs[h][:, c * 512:(c + 1) * 512],
                                         lhsT=vrange(kb, h), rhs=rhs_fn(h, c),
                                         start=start, stop=stop)

            def kloop_main(qh, pvs):
                q0 = qh * SQH
                prev = None
                for kb in range(NKB_HEAD):
                    sc = sc_pool.tile([128, HPC, SQH], f16, tag="sc",
                                      name=f"sc{qh}_{kb}")
                    for c in range(2):
                        sp = sh_ps.tile([128, HPC, 512], f32, tag="sp",
                                        name=f"sp{qh}_{kb}_{c}")
                        for h in range(HPC):
                            nc.tensor.matmul(
                                out=sp[:, h, :],
                                lhsT=KT[h * DH:(h + 1) * DH,
                                        kb * 128:(kb + 1) * 128],
                                rhs=QT[h * DH:(h + 1) * DH,
                                       q0 + c * 512:q0 + (c + 1) * 512],
                                start=True, stop=True)
                        # one exp over both heads halves the ACT op/sem count
                        nc.scalar.activation(
                            out=sc[:, :, c * 512:(c + 1) * 512],
                            in_=sp, func=AF.Exp)
                    if prev is not None:
                        pkb, psc = prev
                        pv_mm(pvs, pkb, lambda h, c, t=psc:
                              t[:, h, c * 512:(c + 1) * 512],
                              start=(pkb == 0), stop=False)
                    prev = (kb, sc)
                pkb, psc = prev
                pv_mm(pvs, pkb, lambda h, c, t=psc: t[:, h, c * 512:(c + 1) * 512],
                      start=False, stop=True)
                # evacuate the main-block partial now; the 2 tail k-blocks are
                # added into hoT later (frees PSUM so both q-halves' main loops
                # run back-to-back while the cope chains finish on DVE/GPSIMD).
                # ACT, not DVE: the DVE queue is full of chain work here and a
                # queued copy would stall the next half's PSUM allocation.
                for h in range(HPC):
                    nc.scalar.activation(out=hoT[h][:, q0:q0 + SQH], in_=pvs[h],
                                         func=AF.Identity)

            def kloop_tail(qh):
                q0 = qh * SQH
                pvt = [sh_ps.tile([65, SQH], f32, tag="sp", name=f"pvt{qh}_{h}")
                       for h in range(HPC)]
                for kb in (NKB_HEAD, NKB_HEAD + 1):
                    t = kb - NKB_HEAD

                    def rhs_fn(h, c, t=t):
                        v = sct[h][:, t, qh * 8:(qh + 1) * 8, :]
                        return v.rearrange("p a b -> p (a b)")[:,
                                                              c * 512:(c + 1) * 512]
                    pv_mm(pvt, kb, rhs_fn, start=(kb == NKB_HEAD),
                          stop=(kb == NKB - 1))
                for h in range(HPC):
                    nc.vector.tensor_tensor(out=hoT[h][:, q0:q0 + SQH],
                                            in0=hoT[h][:, q0:q0 + SQH],
                                            in1=pvt[h], op=OP.add)

            def tail_exp_transpose():
                for h in range(HPC):
                    for qg in range(2):
                        ptq = sh_ps.tile([128, 8, 2, 128], f16, tag="sp",
                                         name=f"sctp{h}_{qg}")
                        for qq in range(8):
                            qb = qg * 8 + qq
                            etile = et.tile([128, TAIL], f16, tag="etile",
                                            name=f"et{h}_{qb}")
                            nc.scalar.activation(out=etile, in_=ttmp[:, h, qb, :],
                                                 func=AF.Exp)
                            for t in range(2):
                                nc.tensor.transpose(
                                    ptq[:, qq, t, :],
                                    etile[:, t * 128:(t + 1) * 128], identh)
                        nc.vector.tensor_copy(
                            out=sct[h][:, :, qg * 8:(qg + 1) * 8, :],
                            in_=ptq.rearrange("p q t c -> p t q c"))

            def dn(qh):
                q0 = qh * SQH
                dcol = dc_pool.tile([128, HPC, 8, 1], f32, tag="dcol",
                                    name=f"dcol{qh}")
                for h in range(HPC):
                    # 66-wide sub-tiles keep each transpose 4-byte aligned
                    ptd = sh_ps.tile([128, 8, 66], f16, tag="sp",
                                     name=f"dnp{qh}_{h}")
                    for sb in range(8):
                        nc.tensor.transpose(
                            ptd[:, sb, 0:65],
                            hoT[h][:, q0 + sb * 128:q0 + (sb + 1) * 128],
                            identh[0:65, 0:65])
                    nc.vector.tensor_copy(out=dcol[:, h], in_=ptd[:, :, 64:65])
                nc.vector.reciprocal(
                    out=rden[:, :, qh * 8:(qh + 1) * 8],
                    in_=dcol.rearrange("p h s o -> p h (s o)"))

            def wo(qh):
                q0 = qh * SQH
                for sb in range(8):
                    sbq = qh * 8 + sb
                    for c in range(2):
                        wop = sh_ps.tile([128, HPC, 512], f32, tag="sp",
                                         name=f"wop{qh}_{sb}_{c}")
                        for h in range(HPC):
                            nc.tensor.matmul(
                                out=wop[:, h, :],
                                lhsT=hoT[h][0:64,
                                            q0 + sb * 128:q0 + (sb + 1) * 128],
                                rhs=woT[h][:, c * 512:(c + 1) * 512],
                                start=True, stop=True)
                        ob0 = ob_pool.tile([128, 512], f32, tag="ob",
                                           name=f"ob{qh}_{sb}_{c}")
                        nc.scalar.activation(out=ob0, in_=wop[:, 0, :],
                                             func=AF.Identity,
                                             scale=rden[:, 0, sbq:sbq + 1])
                        obf = ob_pool.tile([128, 512], f16, tag="obf",
                                           name=f"obf{qh}_{sb}_{c}")
                        nc.vector.scalar_tensor_tensor(
                            out=obf, in0=wop[:, 1, :],
                            scalar=rden[:, 1, sbq:sbq + 1],
                            in1=ob0, op0=OP.mult, op1=OP.add)
                        nc.sync.dma_start(
                            out=out_d[sbq * 128:(sbq + 1) * 128,
                                      c * 512:(c + 1) * 512],
                            in_=obf)

            pv0 = [pv_ps.tile([65, SQH], f32, tag=f"pv{h}", name=f"pv0_{h}")
                   for h in range(HPC)]
            kloop_main(0, pv0)
            pv1 = [pv_ps.tile([65, SQH], f32, tag=f"pv{h}", name=f"pv1_{h}")
                   for h in range(HPC)]
            kloop_main(1, pv1)
            tail_exp_transpose()
            kloop_tail(0)
            dn(0)
            wo(0)
            kloop_tail(1)
            dn(1)
            wo(1)

        nc.sync.dma_start(out=flag_d,
                          in_=flag_sb.rearrange("p h q -> p (h q)"))
        big.release()
        consts.release()

    nc.compile()
    return nc


def bass_ap_2range(Vn, kb):
    """out AP [128, 2, 64] covering Vn[:, kb, 0:64] and Vn[:, kb, 66:130]."""
    import concourse.bass as bass
    base = Vn[:, kb, 0:64]
    ap = [list(base.ap[0]), [66, 2], [1, 64]]
    return bass.AP(base.tensor, base.offset, [list(p) for p in ap])


def _prepare_maps(q, k, v, Wq_w, Wq_b, Wk_w, Wk_b, Wv_w, Wv_b, Wo_w, Wo_b,
                  pos_emb):
    f16, f32 = np.float16, np.float32
    qT = np.ascontiguousarray(q[0].T.astype(f16))
    kT = np.ascontiguousarray(k[0].T.astype(f16))
    vT = np.ascontiguousarray(v[0].T.astype(f16))

    def warr(w):
        # [1024, 128] -> [128, 8*128] partition-major for contiguous DMA
        return np.ascontiguousarray(
            w.astype(f16).reshape(8, 128, DHC).transpose(1, 0, 2).reshape(
                128, 8 * DHC))
    n = HPC * GRP * TAILC
    iota = np.tile(np.arange(1, n + 1, dtype=np.int16), (128, 1))
    # block (h, j) of the merged group gets table offset (h*GRP + j) * NPOS
    off = np.tile(
        np.repeat(np.arange(HPC * GRP, dtype=f16) * NPOS, TAILC),
        (128, 1)).reshape(128, HPC, GRP, TAILC)
    rmask = np.ones((128, HPC, GRP, TAILC), f16)
    rmask[:, :, :, 0] = 0.0
    identh = np.eye(128, dtype=f16)
    pe = np.ascontiguousarray(pos_emb.astype(f16))
    in_maps = []
    for c in range(NCORES):
        r0 = c * DHC
        sl = slice(r0, r0 + DHC)
        in_maps.append({
            "qT": qT, "kT": kT, "vT": vT,
            "wqT": warr(Wq_w[sl, :].T),
            "wkT": warr(Wk_w[sl, :].T),
            "wvT": warr(Wv_w[sl, :].T),
            "woT": np.ascontiguousarray(Wo_w[:, sl].T.astype(f16)),
            "bq": np.ascontiguousarray(Wq_b[sl].astype(f32)[:, None]),
            "bk": np.ascontiguousarray((Wk_b[sl] * 0.125).astype(f32)[:, None]),
            "bv": np.ascontiguousarray(Wv_b[sl].astype(f32)[:, None]),
            "pe": np.concatenate([pe, pe], axis=0),
            "iota": iota, "off": np.ascontiguousarray(off),
            "rmask": np.ascontiguousarray(rmask),
            "identh": identh,
        })
    return in_maps


def _reference_fallback(q, k, v, Wq_w, Wq_b, Wk_w, Wk_b, Wv_w, Wv_b, Wo_w, Wo_b,
                        pos_emb, nheads):
    """Exact numpy fallback (used only if the clip-safety flag fails)."""
    b, s, ndims = q.shape
    d = ndims // nheads

    def heads(x, W, bb):
        y = x.reshape(-1, ndims) @ W.T + bb
        return y.reshape(b, s, nheads, d).transpose(0, 2, 1, 3)

    Q = heads(q, Wq_w, Wq_b)
    K = heads(k, Wk_w, Wk_b)
    V = heads(v, Wv_w, Wv_b)
    logits = np.einsum("bhqd,bhkd->bhqk", Q, K) / np.sqrt(d)
    npos = pos_emb.shape[-1]
    gates = 1.0 / (1.0 + np.exp(-logits))
    pos = np.flip(np.cumsum(np.flip(gates, -1), -1), -1)
    pos = np.minimum(pos, npos - 1)
    pc = np.ceil(pos).astype(np.int64)
    pf = np.floor(pos).astype(np.int64)
    li = np.einsum("bhqd,dn->bhqn", Q, pos_emb)
    lc = np.take_along_axis(li, pc, -1)
    lf = np.take_along_axis(li, pf, -1)
    w = pos - pf
    cope = lc * w + lf * (1.0 - w)
    x = logits + cope
    x = x - x.max(-1, keepdims=True)
    e = np.exp(x)
    scores = e / e.sum(-1, keepdims=True)
    out = np.einsum("bhqk,bhkd->bhqd", scores, V)
    out = out.transpose(0, 2, 1, 3).reshape(b, s, ndims)
    return (out @ Wo_w.T + Wo_b).astype(np.float32)


def kernel(q, k, v, Wq_w, Wq_b, Wk_w, Wk_b, Wv_w, Wv_b, Wo_w, Wo_b, pos_emb,
           nheads, _want_trace=False):
    global _prog
    from concourse.bass_utils import run_bass_kernel_spmd

    q = np.asarray(q); k = np.asarray(k); v = np.asarray(v)
    Wq_w = np.asarray(Wq_w); Wq_b = np.asarray(Wq_b)
    Wk_w = np.asarray(Wk_w); Wk_b = np.asarray(Wk_b)
    Wv_w = np.asarray(Wv_w); Wv_b = np.asarray(Wv_b)
    Wo_w = np.asarray(Wo_w); Wo_b = np.asarray(Wo_b)
    pos_emb = np.asarray(pos_emb)

    if _prog is None:
        _prog = _build_program()
    in_maps = _prepare_maps(q, k, v, Wq_w, Wq_b, Wk_w, Wk_b, Wv_w, Wv_b,
                            Wo_w, Wo_b, pos_emb)
    res = run_bass_kernel_spmd(_prog, in_maps, core_ids=list(range(NCORES)),
                               trace=_want_trace)
    flag_min = min(float(r["flag"].min()) for r in res.results)
    if flag_min < float(NPOS - 1):
        out = _reference_fallback(q, k, v, Wq_w, Wq_b, Wk_w, Wk_b, Wv_w, Wv_b,
                                  Wo_w, Wo_b, pos_emb, int(nheads))
        return out if not _want_trace else (out, res)
    total = res.results[0]["out"].astype(np.float64)
    for r in res.results[1:]:
        total = total + r["out"].astype(np.float64)
    out = (total + Wo_b.astype(np.float64)).astype(np.float32)[None]
    return out if not _want_trace else (out, res)


# revision 21
# speedup vs baseline: 1.1832x; 1.1832x over previous
"""CoPE multi-head attention Trainium2 kernel (v2).

Sharding: 16 heads / 8 cores = 2 heads per core (head/tensor parallel).
Each core gets full q,k,v (host-transposed, fp16) + its head-slice of the
projection weights, computes both heads' attention and the summed output
projection partial; host sums the 8 fp16 partials and adds the output bias.

CoPE structure exploited: pos = reverse-cumsum of sigmoid gates clips at
npos-1=63.  For keys before a 256-wide tail suffix, pos >= 63 (verified at
runtime via a flag output), so cope == T[q,63], a per-row constant that
cancels in softmax.  Only the tail needs the real interpolated gather,
done via GPSIMD local_scatter (run-start positions -> table values) + a
sample-and-hold tensor_tensor_scan.

v2 vs v1: fp16 data everywhere (half DMA, FWL weight loads, DVE 2x),
row-tile-packed 64-contract matmuls (both heads concurrent in the PE
array), merged two-head tail chains (half the DVE op count), q-halved
k-loop fitting PSUM, on-device head-sum + fp16 output (1/4 the output
DMA), engine rebalancing (exp stream owns ACT, chains own DVE/GPSIMD).
"""

import numpy as np

B, S, ND, NH, DH, NPOS = 1, 2048, 1024, 16, 64, 64
NCORES = 8
HPC = NH // NCORES          # heads per core = 2
DHC = HPC * DH              # head dims per core = 128
TAIL = 256                  # tail width (suffix of key axis)
KHEAD = S - TAIL            # 1792
NQB = S // 128              # 16 q blocks
NKB = S // 128              # 16 k blocks
NKB_HEAD = KHEAD // 128     # 14
GRP = 4                     # q-blocks per merged scatter group
NG = NQB // GRP             # 4 groups
TAILC = 160                 # columns of the tail that get the full CoPE chain
TAIL0 = TAIL - TAILC        # leading tail columns treated as clipped
SQH = S // 2                # q-half width = 1024

_prog = None


def _build_program():
    import concourse.bacc as bacc
    import concourse.tile as tile
    from concourse import mybir

    dt = mybir.dt
    AF = mybir.ActivationFunctionType
    OP = mybir.AluOpType
    f16 = dt.float16
    f32 = dt.float32

    nc = bacc.Bacc("TRN2", target_bir_lowering=False, debug=False,
                   num_devices=NCORES)

    # ---- DRAM I/O ----
    qT_d = nc.dram_tensor("qT", [ND, S], f16, kind="ExternalInput").ap()
    kT_d = nc.dram_tensor("kT", [ND, S], f16, kind="ExternalInput").ap()
    vT_d = nc.dram_tensor("vT", [ND, S], f16, kind="ExternalInput").ap()
    # host pre-arranges projection weights as [128, 8, 128] (partition-major)
    # so the DMA is one contiguous 2KB line per partition
    wqT_d = nc.dram_tensor("wqT", [128, 8 * DHC], f16, kind="ExternalInput").ap()
    wkT_d = nc.dram_tensor("wkT", [128, 8 * DHC], f16, kind="ExternalInput").ap()
    wvT_d = nc.dram_tensor("wvT", [128, 8 * DHC], f16, kind="ExternalInput").ap()
    woT_d = nc.dram_tensor("woT", [DHC, ND], f16, kind="ExternalInput").ap()
    bq_d = nc.dram_tensor("bq", [DHC, 1], f32, kind="ExternalInput").ap()
    bk_d = nc.dram_tensor("bk", [DHC, 1], f32, kind="ExternalInput").ap()  # pre-scaled 1/8
    bv_d = nc.dram_tensor("bv", [DHC, 1], f32, kind="ExternalInput").ap()
    pe_d = nc.dram_tensor("pe", [2 * DH, NPOS], f16, kind="ExternalInput").ap()
    iota_d = nc.dram_tensor("iota", [128, HPC * GRP * TAILC], dt.int16,
                            kind="ExternalInput").ap()
    off_d = nc.dram_tensor("off", [128, HPC, GRP, TAILC], f16,
                           kind="ExternalInput").ap()
    rmask_d = nc.dram_tensor("rmask", [128, HPC, GRP, TAILC], f16,
                             kind="ExternalInput").ap()
    identh_d = nc.dram_tensor("identh", [128, 128], f16, kind="ExternalInput").ap()
    out_d = nc.dram_tensor("out", [S, ND], f16, kind="ExternalOutput").ap()
    flag_d = nc.dram_tensor("flag", [128, HPC * NQB], f32,
                            kind="ExternalOutput").ap()

    with tile.TileContext(nc) as tc:
        consts = tc.alloc_tile_pool(name="consts", bufs=1)
        big = tc.alloc_tile_pool(name="big", bufs=1)

        # ---- constants ----
        wqT = consts.tile([128, 8, DHC], f16, tag="wq")
        wkT = consts.tile([128, 8, DHC], f16, tag="wk")
        wvT = consts.tile([128, 8, DHC], f16, tag="wv")
        nc.sync.dma_start(out=wqT, in_=wqT_d.rearrange("p (b d) -> p b d", b=8))
        nc.sync.dma_start(out=wkT, in_=wkT_d.rearrange("p (b d) -> p b d", b=8))
        nc.sync.dma_start(out=wvT, in_=wvT_d.rearrange("p (b d) -> p b d", b=8))
        woT = [consts.tile([DH, ND], f16, tag=f"wo{h}", name=f"woT{h}")
               for h in range(HPC)]
        for h in range(HPC):
            nc.sync.dma_start(out=woT[h], in_=woT_d[h * DH:(h + 1) * DH, :])
        bq = consts.tile([DHC, 1], f32, tag="bq")
        bk = consts.tile([DHC, 1], f32, tag="bk")
        bv = consts.tile([DHC, 1], f32, tag="bv")
        nc.sync.dma_start(out=bq, in_=bq_d)
        nc.sync.dma_start(out=bk, in_=bk_d)
        nc.sync.dma_start(out=bv, in_=bv_d)
        pe = consts.tile([2 * DH, NPOS], f16, tag="pe")
        nc.sync.dma_start(out=pe, in_=pe_d)
        iota = consts.tile([128, HPC * GRP * TAILC], dt.int16, tag="iota")
        nc.sync.dma_start(out=iota, in_=iota_d)
        offt = consts.tile([128, HPC, GRP, TAILC], f16, tag="offt")
        nc.sync.dma_start(out=offt, in_=off_d)
        rmask = consts.tile([128, HPC, GRP, TAILC], f16, tag="rmask")
        nc.sync.dma_start(out=rmask, in_=rmask_d)
        identh = consts.tile([128, 128], f16, tag="identh")
        nc.sync.dma_start(out=identh, in_=identh_d)

        # ---- persistent activations ----
        QT = big.tile([DHC, S], f16, tag="QT")   # h0 rows 0:64, h1 rows 64:128
        KT = big.tile([DHC, S], f16, tag="KT")   # pre-scaled by 1/8
        # per kblk: h0 V[0:64], ones col 64, h1 V cols 66:130, ones col 130
        Vn = big.tile([128, NKB, 131], f16, tag="Vn")
        # cope tables, group-major so a merged group slice is contiguous
        Tp = big.tile([128, NG, HPC, GRP, NPOS], f16, tag="Tp")
        dTp = big.tile([128, NG, HPC, GRP, NPOS], f16, tag="dTp")
        # raw tail logits, then logits+cope (pre-exp)
        ttmp = big.tile([128, HPC, NQB, TAIL], f16, tag="ttmp")
        # transposed tail exp-scores per head: [t-block, qb, q]
        sct = [big.tile([128, 2, NQB, 128], f16, tag=f"sct{h}", name=f"sct{h}")
               for h in range(HPC)]
        hoT = [big.tile([65, S], f16, tag=f"hoT{h}", name=f"hoT{h}")
               for h in range(HPC)]
        rden = big.tile([128, HPC, NQB], f32, tag="rden")
        flag_sb = big.tile([128, HPC, NQB], f32, tag="flag")

        with tc.tile_pool(name="xstage", bufs=3) as xstage, \
             tc.tile_pool(name="proj_ps", bufs=1, space="PSUM") as proj_ps, \
             tc.tile_pool(name="tl1", bufs=1) as tl1, \
             tc.tile_pool(name="tl2", bufs=2) as tl2, \
             tc.tile_pool(name="tl3", bufs=3) as tl3, \
             tc.tile_pool(name="tls", bufs=4) as tls:

            # ---- phase 1: Q,K projections ----
            for name, x_d, wT, bias, scale in (
                ("q", qT_d, wqT, bq, 1.0),
                ("k", kT_d, wkT, bk, 0.125),
            ):
                dest = QT if name == "q" else KT
                ps_chunks = [proj_ps.tile([DHC, 512], f32, tag=f"pp{c}",
                                          name=f"pp_{name}_{c}") for c in range(4)]
                for db in range(8):
                    xt = xstage.tile([128, S], f16, tag="xt", name=f"xt_{name}_{db}")
                    nc.sync.dma_start(out=xt, in_=x_d[db * 128:(db + 1) * 128, :])
                    for c in range(4):
                        nc.tensor.matmul(out=ps_chunks[c], lhsT=wT[:, db, :],
                                         rhs=xt[:, c * 512:(c + 1) * 512],
                                         start=(db == 0), stop=(db == 7))
                for c in range(4):
                    nc.scalar.activation(out=dest[:, c * 512:(c + 1) * 512],
                                         in_=ps_chunks[c], func=AF.Identity,
                                         bias=bias, scale=scale)

            # ---- phase 2: cope tables T'[q,n], dT'[q,n] (fp16), packed heads ----
            with tc.tile_pool(name="tt_ps", bufs=2, space="PSUM") as tt_ps, \
                 tc.tile_pool(name="tsbp", bufs=2) as tsbp:
                for gg in range(2):
                    ps = tt_ps.tile([128, HPC, 8, NPOS], f32, tag="ttp")
                    for qq in range(8):
                        qb = gg * 8 + qq
                        for h in range(HPC):
                            nc.tensor.matmul(
                                out=ps[:, h, qq, :],
                                lhsT=QT[h * DH:(h + 1) * DH, qb * 128:(qb + 1) * 128],
                                rhs=pe[h * DH:(h + 1) * DH, :],
                                start=True, stop=True)
                    tsb = tsbp.tile([128, HPC, 8, NPOS], f32, tag="tsb")
                    nc.vector.tensor_copy(out=tsb, in_=ps)
                    for qq in range(8):
                        qb = gg * 8 + qq
                        g, j = qb // GRP, qb % GRP
                        for h in range(HPC):
                            nc.vector.tensor_scalar(
                                out=Tp[:, g, h, j, :], in0=tsb[:, h, qq, :],
                                scalar1=tsb[:, h, qq, 63:64], scalar2=None,
                                op0=OP.subtract)
                    for h in range(HPC):
                        nc.vector.tensor_tensor(
                            out=dTp[:, gg * 2:(gg + 1) * 2, h, :, 0:63],
                            in0=tsb[:, h, :, 1:64].rearrange(
                                "p (g j) n -> p g j n", g=2),
                            in1=tsb[:, h, :, 0:63].rearrange(
                                "p (g j) n -> p g j n", g=2),
                            op=OP.subtract)
                nc.vector.memset(dTp[:, :, :, :, 63:64], 0.0)

            # ---- phase 3: tail QK (packed heads) -> raw logits into ttmp ----
            with tc.tile_pool(name="st_ps", bufs=1, space="PSUM") as st_ps:
                for qb in range(NQB):
                    stp = st_ps.tile([128, HPC, 512], f32, tag="stp")
                    for h in range(HPC):
                        nc.tensor.matmul(
                            out=stp[:, h, 0:TAIL],
                            lhsT=QT[h * DH:(h + 1) * DH, qb * 128:(qb + 1) * 128],
                            rhs=KT[h * DH:(h + 1) * DH, KHEAD:S],
                            start=True, stop=True)
                    nc.scalar.activation(out=ttmp[:, :, qb, :],
                                         in_=stp[:, :, 0:TAIL], func=AF.Identity)

            # ---- phase 4: merged two-head tail cope chains ----
            def chain_pre(g):
                gts = tl2.tile([128, HPC, GRP, TAILC], f16, tag="gts",
                               name=f"gts{g}")
                posb = tl2.tile([128, HPC, GRP, TAILC], f32, tag="posb",
                                name=f"posb{g}")
                wb = tl3.tile([128, HPC, GRP, TAILC], f16, tag="wb",
                              name=f"wb{g}")
                c16 = tl2.tile([128, HPC, GRP, TAILC], dt.int16, tag="c16",
                               name=f"c16{g}")
                cfb = tl2.tile([128, HPC, GRP, TAILC], f16, tag="cfb",
                               name=f"cfb{g}")
                ieq = tl3.tile([128, HPC, GRP, TAILC], f16, tag="ieq",
                               name=f"ieq{g}")
                mi16 = tl3.tile([128, HPC, GRP, TAILC], dt.int16, tag="mi16",
                                name=f"mi16{g}")
                gsb = tls.tile([128, HPC, GRP], f32, tag="gsb", name=f"gsb{g}")
                spsc = ttmp[:, :, g * GRP:(g + 1) * GRP, TAIL0:TAIL]
                for h in range(HPC):
                    for j in range(GRP):
                        nc.scalar.activation(
                            out=gts[:, h, j, :],
                            in_=ttmp[:, h, g * GRP + j, TAIL0:TAIL],
                            func=AF.Sigmoid, accum_out=gsb[:, h, j:j + 1])
                nc.vector.tensor_copy(out=flag_sb[:, :, g * GRP:(g + 1) * GRP],
                                      in_=gsb)
                # col0 of each sub-block: g[0] - gsum (seeds the chained scan)
                nc.vector.tensor_tensor(
                    out=gts[:, :, :, 0:1], in0=gts[:, :, :, 0:1],
                    in1=gsb.rearrange("p h (g o) -> p h g o", o=1), op=OP.subtract)
                # chained cumsum with reset: state = rmask*state + g'
                nc.vector.tensor_tensor_scan(
                    out=posb.rearrange("p h g t -> p (h g t)"),
                    data0=rmask.rearrange("p h g t -> p (h g t)"),
                    data1=gts.rearrange("p h g t -> p (h g t)"), initial=0.0,
                    op0=OP.mult, op1=OP.add)
                # restore col0, then pos = g - (cs - gsum)
                nc.vector.tensor_tensor(
                    out=gts[:, :, :, 0:1], in0=gts[:, :, :, 0:1],
                    in1=gsb.rearrange("p h (g o) -> p h g o", o=1), op=OP.add)
                nc.vector.scalar_tensor_tensor(out=posb, in0=posb, scalar=-1.0,
                                               in1=gts, op0=OP.mult, op1=OP.add)
                nc.vector.tensor_scalar(out=c16, in0=posb, scalar1=0.5,
                                        scalar2=62.0, op0=OP.subtract, op1=OP.min)
                nc.vector.tensor_copy(out=cfb, in_=c16)
                nc.vector.scalar_tensor_tensor(out=wb, in0=posb, scalar=63.0,
                                               in1=cfb, op0=OP.min, op1=OP.subtract)
                # ieq = 1 where floor-index unchanged (keep-state mask for the
                # sample-and-hold scans); 0 at run starts.  col0 forced run-start.
                nc.vector.tensor_tensor(out=ieq[:, :, :, 1:], in0=cfb[:, :, :, 1:],
                                        in1=cfb[:, :, :, :-1], op=OP.is_equal)
                nc.vector.memset(ieq[:, :, :, 0:1], 0.0)
                # scatter index: cfb+off at run starts, pushed negative elsewhere
                nc.vector.tensor_add(out=cfb, in0=cfb, in1=offt)
                nc.vector.scalar_tensor_tensor(out=mi16, in0=ieq, scalar=-1024.0,
                                               in1=cfb, op0=OP.mult, op1=OP.add)
                return dict(gts=gts, posb=posb, wb=wb, c16=c16, cfb=cfb, ieq=ieq,
                            mi16=mi16, gsb=gsb, spsc=spsc)

            def chain_scatter1(g, st):
                ptab = tl3.tile([128, HPC * GRP * NPOS], dt.int16, tag="ptab",
                                name=f"ptab{g}")
                nc.gpsimd.local_scatter(out_ap=ptab[:], data_ap=iota[:],
                                        idxs_ap=st['mi16'][:], channels=128,
                                        num_elems=HPC * GRP * NPOS,
                                        num_idxs=HPC * GRP * TAILC)
                st['ptab'] = ptab

            def chain_scatter23(g, st):
                ptab = st['ptab']
                scA = tl3.tile([128, HPC, GRP, TAILC], f16, tag="scA",
                               name=f"scA{g}")
                scB = tl3.tile([128, HPC, GRP, TAILC], f16, tag="scB",
                               name=f"scB{g}")
                nc.vector.tensor_scalar(out=ptab, in0=ptab, scalar1=1,
                                        scalar2=None, op0=OP.subtract)
                nc.gpsimd.local_scatter(out_ap=scA[:], data_ap=Tp[:, g],
                                        idxs_ap=ptab[:], channels=128,
                                        num_elems=HPC * GRP * TAILC,
                                        num_idxs=HPC * GRP * NPOS)
                nc.gpsimd.local_scatter(out_ap=scB[:], data_ap=dTp[:, g],
                                        idxs_ap=ptab[:], channels=128,
                                        num_elems=HPC * GRP * TAILC,
                                        num_idxs=HPC * GRP * NPOS)
                st['scA'] = scA
                st['scB'] = scB

            def chain_post(g, st):
                Ab = tl1.tile([128, HPC, GRP, TAILC], f16, tag="Ab",
                              name=f"Ab{g}")
                Bb = tl1.tile([128, HPC, GRP, TAILC], f16, tag="Bb",
                              name=f"Bb{g}")
                # col0 of every sub-block is a forced run-start, so one chained
                # scan over the flattened group self-resets at block boundaries
                nc.vector.tensor_tensor_scan(
                    out=Ab.rearrange("p h g t -> p (h g t)"),
                    data0=st['ieq'].rearrange("p h g t -> p (h g t)"),
                    data1=st['scA'].rearrange("p h g t -> p (h g t)"), initial=0.0,
                    op0=OP.mult, op1=OP.add)
                nc.vector.tensor_tensor_scan(
                    out=Bb.rearrange("p h g t -> p (h g t)"),
                    data0=st['ieq'].rearrange("p h g t -> p (h g t)"),
                    data1=st['scB'].rearrange("p h g t -> p (h g t)"), initial=0.0,
                    op0=OP.mult, op1=OP.add)
                nc.vector.tensor_mul(out=Bb, in0=st['wb'], in1=Bb)
                nc.vector.tensor_add(out=Ab, in0=Ab, in1=Bb)
                nc.vector.tensor_add(out=st['spsc'], in0=st['spsc'], in1=Ab)

            # software-pipelined emission: the DVE queue is in-order, so
            # interleave the next groups' pre-work between the GPSIMD scatter
            # calls to keep both engines busy (a cross-engine wait parked at
            # the DVE queue head would otherwise idle it for the whole scatter)
            sts = [None] * NG
            sts[0] = chain_pre(0)
            chain_scatter1(0, sts[0])
            if NG > 1:
                sts[1] = chain_pre(1)
            for g in range(NG):
                chain_scatter23(g, sts[g])
                if g + 1 < NG:
                    chain_scatter1(g + 1, sts[g + 1])
                if g + 2 < NG:
                    sts[g + 2] = chain_pre(g + 2)
                chain_post(g, sts[g])

            # ---- phase 5: V projection + transpose-pack into Vn ----
            with tc.tile_pool(name="vt_ps", bufs=2, space="PSUM") as vt_ps, \
                 tc.tile_pool(name="vstage", bufs=1) as vstage:
                VT = vstage.tile([DHC, S], f16, tag="VT")
                ps_chunks = [proj_ps.tile([DHC, 512], f32, tag=f"pp{c}",
                                          name=f"pp_v_{c}") for c in range(4)]
                for db in range(8):
                    xt = xstage.tile([128, S], f16, tag="xt", name=f"xt_v_{db}")
                    nc.sync.dma_start(out=xt, in_=vT_d[db * 128:(db + 1) * 128, :])
                    for c in range(4):
                        nc.tensor.matmul(out=ps_chunks[c], lhsT=wvT[:, db, :],
                                         rhs=xt[:, c * 512:(c + 1) * 512],
                                         start=(db == 0), stop=(db == 7))
                for c in range(4):
                    nc.scalar.activation(out=VT[:, c * 512:(c + 1) * 512],
                                         in_=ps_chunks[c], func=AF.Identity,
                                         bias=bv)
                nc.vector.memset(Vn[:, :, 64:65], 1.0)
                nc.vector.memset(Vn[:, :, 130:131], 1.0)
                for kb in range(NKB):
                    pt = vt_ps.tile([128, 128], f16, tag="vt")
                    nc.tensor.transpose(pt, VT[:, kb * 128:(kb + 1) * 128], identh)
                    dst = bass_ap_2range(Vn, kb)
                    nc.scalar.activation(out=dst,
                                         in_=pt.rearrange("p (b d) -> p b d", b=2),
                                         func=AF.Identity)

        # ---- phases 6-9: k-loop (both heads packed, q-halved), tail exp +
        # transpose, denominators, output projection with on-device head-sum ----
        # One shared 2-slot PSUM pool (tag "sp", [128,2,512]f32-sized = 2 banks
        # per slot) serves the k-loop QK tiles, the tail transposes, the
        # denominator transposes and the Wo accumulators: pv(4) + shared(4) = 8.
        with tc.tile_pool(name="pv_ps", bufs=1, space="PSUM") as pv_ps, \
             tc.tile_pool(name="sh_ps", bufs=2, space="PSUM") as sh_ps, \
             tc.tile_pool(name="sc_pool", bufs=3) as sc_pool, \
             tc.tile_pool(name="et", bufs=2) as et, \
             tc.tile_pool(name="ob_pool", bufs=2) as ob_pool, \
             tc.tile_pool(name="dc_pool", bufs=2) as dc_pool:

            def vrange(kb, h):
                return Vn[:, kb, 0:65] if h == 0 else Vn[:, kb, 66:131]

            def pv_mm(pvs, kb, rhs_fn, start, stop):
                for h in range(HPC):
                    for c in range(2):
                        nc.tensor.matmul(out=pvs[h][:, c * 512:(c + 1) * 512],
                                         lhsT=vrange(kb, h), rhs=rhs_fn(h, c),
                                         start=start, stop=stop)

            def kloop_main(qh, pvs):
                q0 = qh * SQH
                prev = None
                for kb in range(NKB_HEAD):
                    sc = sc_pool.tile([128, HPC, SQH], f16, tag="sc",
                                      name=f"sc{qh}_{kb}")
                    for c in range(2):
                        sp = sh_ps.tile([128, HPC, 512], f32, tag="sp",
                                        name=f"sp{qh}_{kb}_{c}")
                        for h in range(HPC):
                            nc.tensor.matmul(
                                out=sp[:, h, :],
                                lhsT=KT[h * DH:(h + 1) * DH,
                                        kb * 128:(kb + 1) * 128],
                                rhs=QT[h * DH:(h + 1) * DH,
                                       q0 + c * 512:q0 + (c + 1) * 512],
                                start=True, stop=True)
                        # one exp over both heads halves the ACT op/sem count
                        nc.scalar.activation(
                            out=sc[:, :, c * 512:(c + 1) * 512],
                            in_=sp, func=AF.Exp)
                    if prev is not None:
                        pkb, psc = prev
                        pv_mm(pvs, pkb, lambda h, c, t=psc:
                              t[:, h, c * 512:(c + 1) * 512],
                              start=(pkb == 0), stop=False)
                    prev = (kb, sc)
                pkb, psc = prev
                pv_mm(pvs, pkb, lambda h, c, t=psc: t[:, h, c * 512:(c + 1) * 512],
                      start=False, stop=True)
                # evacuate the main-block partial now; the 2 tail k-blocks are
                # added into hoT later (frees PSUM so both q-halves' main loops
                # run back-to-back while the cope chains finish on DVE/GPSIMD).
                # ACT, not DVE: the DVE queue is full of chain work here and a
                # queued copy would stall the next half's PSUM allocation.
                for h in range(HPC):
                    nc.scalar.activation(out=hoT[h][:, q0:q0 + SQH], in_=pvs[h],
                                         func=AF.Identity)

            def kloop_tail(qh):
                q0 = qh * SQH
                pvt = [sh_ps.tile([65, SQH], f32, tag="sp", name=f"pvt{qh}_{h}")
                       for h in range(HPC)]
                for kb in (NKB_HEAD, NKB_HEAD + 1):
                    t = kb - NKB_HEAD

                    def rhs_fn(h, c, t=t):
                        v = sct[h][:, t, qh * 8:(qh + 1) * 8, :]
                        return v.rearrange("p a b -> p (a b)")[:,
                                                              c * 512:(c + 1) * 512]
                    pv_mm(pvt, kb, rhs_fn, start=(kb == NKB_HEAD),
                          stop=(kb == NKB - 1))
                for h in range(HPC):
                    nc.vector.tensor_tensor(out=hoT[h][:, q0:q0 + SQH],
                                            in0=hoT[h][:, q0:q0 + SQH],
                                            in1=pvt[h], op=OP.add)

            def tail_exp_transpose():
                for h in range(HPC):
                    for qg in range(2):
                        ptq = sh_ps.tile([128, 8, 2, 128], f16, tag="sp",
                                         name=f"sctp{h}_{qg}")
                        for qq in range(8):
                            qb = qg * 8 + qq
                            etile = et.tile([128, TAIL], f16, tag="etile",
                                            name=f"et{h}_{qb}")
                            nc.scalar.activation(out=etile, in_=ttmp[:, h, qb, :],
                                                 func=AF.Exp)
                            for t in range(2):
                                nc.tensor.transpose(
                                    ptq[:, qq, t, :],
                                    etile[:, t * 128:(t + 1) * 128], identh)
                        nc.vector.tensor_copy(
                            out=sct[h][:, :, qg * 8:(qg + 1) * 8, :],
                            in_=ptq.rearrange("p q t c -> p t q c"))

            def dn(qh):
                q0 = qh * SQH
                dcol = dc_pool.tile([128, HPC, 8, 1], f32, tag="dcol",
                                    name=f"dcol{qh}")
                for h in range(HPC):
                    # 66-wide sub-tiles keep each transpose 4-byte aligned
                    ptd = sh_ps.tile([128, 8, 66], f16, tag="sp",
                                     name=f"dnp{qh}_{h}")
                    for sb in range(8):
                        nc.tensor.transpose(
                            ptd[:, sb, 0:65],
                            hoT[h][:, q0 + sb * 128:q0 + (sb + 1) * 128],
                            identh[0:65, 0:65])
                    nc.vector.tensor_copy(out=dcol[:, h], in_=ptd[:, :, 64:65])
                nc.vector.reciprocal(
                    out=rden[:, :, qh * 8:(qh + 1) * 8],
                    in_=dcol.rearrange("p h s o -> p h (s o)"))

            def wo(qh):
                q0 = qh * SQH
                for sb in range(8):
                    sbq = qh * 8 + sb
                    for c in range(2):
                        wop = sh_ps.tile([128, HPC, 512], f32, tag="sp",
                                         name=f"wop{qh}_{sb}_{c}")
                        for h in range(HPC):
                            nc.tensor.matmul(
                                out=wop[:, h, :],
                                lhsT=hoT[h][0:64,
                                            q0 + sb * 128:q0 + (sb + 1) * 128],
                                rhs=woT[h][:, c * 512:(c + 1) * 512],
                                start=True, stop=True)
                        ob0 = ob_pool.tile([128, 512], f32, tag="ob",
                                           name=f"ob{qh}_{sb}_{c}")
                        nc.scalar.activation(out=ob0, in_=wop[:, 0, :],
                                             func=AF.Identity,
                                             scale=rden[:, 0, sbq:sbq + 1])
                        obf = ob_pool.tile([128, 512], f16, tag="obf",
                                           name=f"obf{qh}_{sb}_{c}")
                        nc.vector.scalar_tensor_tensor(
                            out=obf, in0=wop[:, 1, :],
                            scalar=rden[:, 1, sbq:sbq + 1],
                            in1=ob0, op0=OP.mult, op1=OP.add)
                        nc.sync.dma_start(
                            out=out_d[sbq * 128:(sbq + 1) * 128,
                                      c * 512:(c + 1) * 512],
                            in_=obf)

            pv0 = [pv_ps.tile([65, SQH], f32, tag=f"pv{h}", name=f"pv0_{h}")
                   for h in range(HPC)]
            kloop_main(0, pv0)
            pv1 = [pv_ps.tile([65, SQH], f32, tag=f"pv{h}", name=f"pv1_{h}")
                   for h in range(HPC)]
            kloop_main(1, pv1)
            tail_exp_transpose()
            kloop_tail(0)
            dn(0)
            wo(0)
            kloop_tail(1)
            dn(1)
            wo(1)

        nc.sync.dma_start(out=flag_d,
                          in_=flag_sb.rearrange("p h q -> p (h q)"))
        big.release()
        consts.release()

    nc.compile()
    return nc


def bass_ap_2range(Vn, kb):
    """out AP [128, 2, 64] covering Vn[:, kb, 0:64] and Vn[:, kb, 66:130]."""
    import concourse.bass as bass
    base = Vn[:, kb, 0:64]
    ap = [list(base.ap[0]), [66, 2], [1, 64]]
    return bass.AP(base.tensor, base.offset, [list(p) for p in ap])


def _prepare_maps(q, k, v, Wq_w, Wq_b, Wk_w, Wk_b, Wv_w, Wv_b, Wo_w, Wo_b,
                  pos_emb):
    f16, f32 = np.float16, np.float32
    qT = np.ascontiguousarray(q[0].T.astype(f16))
    kT = np.ascontiguousarray(k[0].T.astype(f16))
    vT = np.ascontiguousarray(v[0].T.astype(f16))

    def warr(w):
        # [1024, 128] -> [128, 8*128] partition-major for contiguous DMA
        return np.ascontiguousarray(
            w.astype(f16).reshape(8, 128, DHC).transpose(1, 0, 2).reshape(
                128, 8 * DHC))
    n = HPC * GRP * TAILC
    iota = np.tile(np.arange(1, n + 1, dtype=np.int16), (128, 1))
    # block (h, j) of the merged group gets table offset (h*GRP + j) * NPOS
    off = np.tile(
        np.repeat(np.arange(HPC * GRP, dtype=f16) * NPOS, TAILC),
        (128, 1)).reshape(128, HPC, GRP, TAILC)
    rmask = np.ones((128, HPC, GRP, TAILC), f16)
    rmask[:, :, :, 0] = 0.0
    identh = np.eye(128, dtype=f16)
    pe = np.ascontiguousarray(pos_emb.astype(f16))
    in_maps = []
    for c in range(NCORES):
        r0 = c * DHC
        sl = slice(r0, r0 + DHC)
        in_maps.append({
            "qT": qT, "kT": kT, "vT": vT,
            "wqT": warr(Wq_w[sl, :].T),
            "wkT": warr(Wk_w[sl, :].T),
            "wvT": warr(Wv_w[sl, :].T),
            "woT": np.ascontiguousarray(Wo_w[:, sl].T.astype(f16)),
            "bq": np.ascontiguousarray(Wq_b[sl].astype(f32)[:, None]),
            "bk": np.ascontiguousarray((Wk_b[sl] * 0.125).astype(f32)[:, None]),
            "bv": np.ascontiguousarray(Wv_b[sl].astype(f32)[:, None]),
            "pe": np.concatenate([pe, pe], axis=0),
            "iota": iota, "off": np.ascontiguousarray(off),
            "rmask": np.ascontiguousarray(rmask),
            "identh": identh,
        })
    return in_maps


def _reference_fallback(q, k, v, Wq_w, Wq_b, Wk_w, Wk_b, Wv_w, Wv_b, Wo_w, Wo_b,
                        pos_emb, nheads):
    """Exact numpy fallback (used only if the clip-safety flag fails)."""
    b, s, ndims = q.shape
    d = ndims // nheads

    def heads(x, W, bb):
        y = x.reshape(-1, ndims) @ W.T + bb
        return y.reshape(b, s, nheads, d).transpose(0, 2, 1, 3)

    Q = heads(q, Wq_w, Wq_b)
    K = heads(k, Wk_w, Wk_b)
    V = heads(v, Wv_w, Wv_b)
    logits = np.einsum("bhqd,bhkd->bhqk", Q, K) / np.sqrt(d)
    npos = pos_emb.shape[-1]
    gates = 1.0 / (1.0 + np.exp(-logits))
    pos = np.flip(np.cumsum(np.flip(gates, -1), -1), -1)
    pos = np.minimum(pos, npos - 1)
    pc = np.ceil(pos).astype(np.int64)
    pf = np.floor(pos).astype(np.int64)
    li = np.einsum("bhqd,dn->bhqn", Q, pos_emb)
    lc = np.take_along_axis(li, pc, -1)
    lf = np.take_along_axis(li, pf, -1)
    w = pos - pf
    cope = lc * w + lf * (1.0 - w)
    x = logits + cope
    x = x - x.max(-1, keepdims=True)
    e = np.exp(x)
    scores = e / e.sum(-1, keepdims=True)
    out = np.einsum("bhqk,bhkd->bhqd", scores, V)
    out = out.transpose(0, 2, 1, 3).reshape(b, s, ndims)
    return (out @ Wo_w.T + Wo_b).astype(np.float32)


def kernel(q, k, v, Wq_w, Wq_b, Wk_w, Wk_b, Wv_w, Wv_b, Wo_w, Wo_b, pos_emb,
           nheads, _want_trace=False):
    global _prog
    from concourse.bass_utils import run_bass_kernel_spmd

    q = np.asarray(q); k = np.asarray(k); v = np.asarray(v)
    Wq_w = np.asarray(Wq_w); Wq_b = np.asarray(Wq_b)
    Wk_w = np.asarray(Wk_w); Wk_b = np.asarray(Wk_b)
    Wv_w = np.asarray(Wv_w); Wv_b = np.asarray(Wv_b)
    Wo_w = np.asarray(Wo_w); Wo_b = np.asarray(Wo_b)
    pos_emb = np.asarray(pos_emb)

    if _prog is None:
        _prog = _build_program()
    in_maps = _prepare_maps(q, k, v, Wq_w, Wq_b, Wk_w, Wk_b, Wv_w, Wv_b,
                            Wo_w, Wo_b, pos_emb)
    res = run_bass_kernel_spmd(_prog, in_maps, core_ids=list(range(NCORES)),
                               trace=_want_trace)
    flag_min = min(float(r["flag"].min()) for r in res.results)
    if flag_min < float(NPOS - 1):
        out = _reference_fallback(q, k, v, Wq_w, Wq_b, Wk_w, Wk_b, Wv_w, Wv_b,
                                  Wo_w, Wo_b, pos_emb, int(nheads))
        return out if not _want_trace else (out, res)
    total = res.results[0]["out"].astype(np.float64)
    for r in res.results[1:]:
        total = total + r["out"].astype(np.float64)
    out = (total + Wo_b.astype(np.float64)).astype(np.float32)[None]
    return out if not _want_trace else (out, res)
